# revision 13
# baseline (speedup 1.0000x reference)
"""PhiMoE sparse MoE block on 8 trn2 cores, expert-parallel + token-sparse.

Each core owns one expert. The reference's slot-index combine makes most
token/expert coefficients exactly zero, so each core routes on all tokens
(fp32), compacts the nonzero-coefficient token list on device (prefix-scan +
local_scatter), gathers just those token rows (capacity 256 per token-half),
runs the expert MLP on the gathered tokens (f32r gate/up, bf16 down), scales
by the combine coefficient, scatters rows back, and a per-half ReduceScatter
sums partials across cores; the host reassembles the shards.
"""

import numpy as np

import concourse.bass as bass
import concourse.mybir as mybir
import concourse.tile as tile
from concourse import bacc, library_config
from concourse.bass_utils import run_bass_kernel_spmd
from concourse.masks import make_identity

N_CORES = 8
B, S, H, F, E = 2, 1024, 1024, 4096, 8
T = B * S              # 2048 tokens
NT = T // 128          # 16 token tiles
NK = H // 128          # 8 contraction tiles over H
NF = F // 128          # 32 f tiles
NHALF = 2
TH = T // NHALF        # 1024 tokens per half
CAP = 256              # gathered-token capacity per half (true counts <= ~160)
NST = (NHALF * CAP) // 128   # 4 gathered slot tiles
JITTER2 = 2 * 0.01
BIG = 1.0e30

dt = mybir.dt
f32 = dt.float32
f32r = dt.float32r
bf16 = dt.bfloat16
X = mybir.AxisListType.X
op = mybir.AluOpType


def _bc(ap):
    """Broadcast a [128, NT] AP to [128, NT, E] via a step-0 trailing dim."""
    return bass.AP(tensor=ap.tensor, offset=ap.offset,
                   ap=[list(ap.ap[0]), list(ap.ap[1]), [0, E]])


def _sparsemixer_batch(nc, smx, s_all, iota_all, esel_all, c_all):
    """Top-2 sparsemixer for all NT token tiles at once.

    s_all [128, NT, E] fp32 logits; writes this core's combine coefficient
    (with the reference's slot-index dense-table behavior) to c_all [128, NT].
    """
    def t3(name):
        return smx.tile([128, NT, E], f32, tag=name, name=name)

    def t2(name):
        return smx.tile([128, NT], f32, tag=name, name=name)

    Exp = mybir.ActivationFunctionType.Exp
    m1 = t2("m1")
    nc.vector.tensor_reduce(m1, s_all, axis=X, op=op.max)
    abs_s = t3("abs_s")
    nc.scalar.activation(abs_s, s_all, mybir.ActivationFunctionType.Abs)
    # thr1 = max(|s|, m1)*2eps ; keep1 = (m1 - s) <= thr1
    thr1 = t3("thr1")
    nc.vector.tensor_max(thr1, abs_s, _bc(m1))
    nc.vector.tensor_scalar_mul(thr1, thr1, JITTER2)
    lhs1 = t3("lhs1")
    nc.vector.tensor_sub(lhs1, _bc(m1), s_all)
    keep1 = t3("keep1")
    nc.vector.tensor_tensor(keep1, lhs1, thr1, op.is_le)
    sub1 = t3("sub1")
    nc.vector.tensor_sub(sub1, s_all, _bc(m1))
    expo = t3("expo")
    nc.scalar.activation(expo, sub1, Exp)
    et1 = t3("et1")
    nc.vector.tensor_mul(et1, expo, keep1)
    e1 = t2("e1")
    nc.vector.tensor_reduce(e1, et1, axis=X, op=op.add)
    mult1 = t2("mult1")
    nc.vector.reciprocal(mult1, e1)
    # first argmax (first index on ties) as onehot
    eq1 = t3("eq1")
    nc.vector.tensor_tensor(eq1, s_all, _bc(m1), op.is_equal)
    cand1 = t3("cand1")
    nc.vector.tensor_mul(cand1, eq1, iota_all)
    i1f = t2("i1f")
    nc.vector.tensor_reduce(i1f, cand1, axis=X, op=op.min)
    oh1 = t3("oh1")
    nc.vector.tensor_tensor(oh1, iota_all, _bc(i1f), op.is_equal)
    # mask first argmax, repeat for second expert
    ms = t3("ms")
    nc.vector.scalar_tensor_tensor(ms, oh1, -BIG, s_all, op0=op.mult, op1=op.add)
    m2 = t2("m2")
    nc.vector.tensor_reduce(m2, ms, axis=X, op=op.max)
    thr2 = t3("thr2")
    nc.vector.tensor_max(thr2, abs_s, _bc(m2))
    nc.vector.tensor_scalar_mul(thr2, thr2, JITTER2)
    lhs2 = t3("lhs2")
    nc.vector.tensor_sub(lhs2, _bc(m2), s_all)
    keep2 = t3("keep2")
    nc.vector.tensor_tensor(keep2, lhs2, thr2, op.is_le)
    sub2 = t3("sub2")
    nc.vector.tensor_sub(sub2, ms, _bc(m2))
    expo2 = t3("expo2")
    nc.scalar.activation(expo2, sub2, Exp)
    et2 = t3("et2")
    nc.vector.tensor_mul(et2, expo2, keep2)
    e2 = t2("e2")
    nc.vector.tensor_reduce(e2, et2, axis=X, op=op.add)
    mult2 = t2("mult2")
    nc.vector.reciprocal(mult2, e2)
    eq2 = t3("eq2")
    nc.vector.tensor_tensor(eq2, ms, _bc(m2), op.is_equal)
    cand2 = t3("cand2")
    nc.vector.tensor_mul(cand2, eq2, iota_all)
    i2f = t2("i2f")
    nc.vector.tensor_reduce(i2f, cand2, axis=X, op=op.min)
    oh2 = t3("oh2")
    nc.vector.tensor_tensor(oh2, iota_all, _bc(i2f), op.is_equal)
    # dense-table slots 0/1 (faithful reference combine), [128, NT] ops
    d0 = t2("d0")
    nc.vector.tensor_mul(d0, oh1[:, :, 0], mult1)
    d0b = t2("d0b")
    nc.vector.tensor_mul(d0b, oh2[:, :, 0], mult2)
    nc.vector.tensor_add(d0, d0, d0b)
    d1 = t2("d1")
    nc.vector.tensor_mul(d1, oh1[:, :, 1], mult1)
    d1b = t2("d1b")
    nc.vector.tensor_mul(d1b, oh2[:, :, 1], mult2)
    nc.vector.tensor_add(d1, d1, d1b)
    # this core's expert columns
    t1 = t3("selw1")
    nc.vector.tensor_mul(t1, oh1, esel_all)
    oh1e = t2("oh1e")
    nc.vector.tensor_reduce(oh1e, t1, axis=X, op=op.add)
    t2_ = t3("selw2")
    nc.vector.tensor_mul(t2_, oh2, esel_all)
    oh2e = t2("oh2e")
    nc.vector.tensor_reduce(oh2e, t2_, axis=X, op=op.add)
    ca = t2("ca")
    nc.vector.tensor_mul(ca, oh1e, d0)
    cb = t2("cb")
    nc.vector.tensor_mul(cb, oh2e, d1)
    nc.vector.tensor_add(c_all, ca, cb)


def _build_kernel(tc, xT, xpad, gw, w1, w3, w2, esel, yshard, rlog):
    nc = tc.nc
    Silu = mybir.ActivationFunctionType.Silu

    from contextlib import ExitStack
    ctx = ExitStack()
    constp = ctx.enter_context(tc.tile_pool(name="constp", bufs=1))
    xtgp = ctx.enter_context(tc.tile_pool(name="xtgp", bufs=1))
    w2bp = ctx.enter_context(tc.tile_pool(name="w2bp", bufs=1))
    htp = ctx.enter_context(tc.tile_pool(name="htp", bufs=1))
    fstage = ctx.enter_context(tc.tile_pool(name="fstage", bufs=3))
    gath = ctx.enter_context(tc.tile_pool(name="gath", bufs=3))
    gsp = ctx.enter_context(tc.tile_pool(name="gsp", bufs=2))
    ostage = ctx.enter_context(tc.tile_pool(name="ostage", bufs=2))
    rstat = ctx.enter_context(tc.tile_pool(name="rstat", bufs=2))
    smx = ctx.enter_context(tc.tile_pool(name="smx", bufs=1))
    cmp_ = ctx.enter_context(tc.tile_pool(name="cmp", bufs=1))
    gpsum = ctx.enter_context(tc.tile_pool(name="gpsum", bufs=2, space="PSUM"))
    upsum = ctx.enter_context(tc.tile_pool(name="upsum", bufs=2, space="PSUM"))
    mpsum = ctx.enter_context(tc.tile_pool(name="mpsum", bufs=4, space="PSUM"))
    dram = ctx.enter_context(tc.tile_pool(name="dram", bufs=1, space="DRAM"))

    # ---- constants ----
    gw_sb = constp.tile([128, NK, E], f32, name="gw_sb")
    nc.sync.dma_start(gw_sb[:], gw.rearrange("(k p) e -> p k e", p=128))
    esel_all = constp.tile([128, NT, E], f32, name="esel_all")
    nc.sync.dma_start(esel_all[:], esel)
    iota_all = constp.tile([128, NT, E], f32, name="iota_all")
    for j in range(E):
        nc.vector.memset(iota_all[:, :, j], float(j - E))
    c_all = constp.tile([128, NT], f32, name="c_all")
    m_all = constp.tile([128, NT], f32, name="m_all")
    ident = constp.tile([128, 128], f32, name="ident")
    make_identity(nc, ident[:])

    # DRAM scratch
    y_h = [dram.tile([TH + 1, H], f32, name=f"y_h{h}") for h in range(NHALF)]
    y_rs = [dram.tile([TH // N_CORES, H], f32, name=f"y_rs{h}")
            for h in range(NHALF)]
    cpad = dram.tile([T + 1, 1], f32, name="cpad")
    md = [dram.tile([1, TH], f32, name=f"md{h}") for h in range(NHALF)]
    idxd = [dram.tile([1, CAP], dt.int32, name=f"idxd{h}") for h in range(NHALF)]

    # zero-fill y halves + cpad row 0 early
    zz = constp.tile([128, H], f32, name="zz")
    nc.vector.memset(zz[:], 0.0)
    for h in range(NHALF):
        for r in range(0, TH + 1, 128):
            nr = min(128, TH + 1 - r)
            nc.sync.dma_start(y_h[h][r:r + nr, :], zz[:nr, :])
    nc.sync.dma_start(cpad[0:1, :], zz[0:1, 0:1])

    # ---- router (fp32, replicated) ----
    s_all = constp.tile([128, NT, E], f32, name="s_all")
    for t in range(NT):
        lg = mpsum.tile([128, E], f32, tag="mp", name=f"lg{t}")
        xst = rstat.tile([128, NK, 128], f32, tag="xst", name=f"xst{t}")
        nc.sync.dma_start(
            xst[:], xT[t].rearrange("p (k c) -> p k c", k=NK))
        for k in range(NK):
            nc.tensor.matmul(lg[:], xst[:, k, :], gw_sb[:, k, :],
                             start=(k == 0), stop=(k == NK - 1))
        nc.scalar.copy(s_all[:, t, :], lg[:])
    nc.sync.dma_start(rlog.rearrange("(t p) e -> p t e", p=128), s_all[:])
    # ---- batched sparsemixer over all tiles ----
    _sparsemixer_batch(nc, smx, s_all[:], iota_all[:], esel_all[:], c_all[:])
    nc.vector.tensor_scalar(m_all[:], c_all[:], 0.0, None, op0=op.not_equal)

    # write coefficients token-major to cpad rows 1..T
    nc.sync.dma_start(
        cpad[1:T + 1, :].rearrange("(t p) a -> p (t a)", p=128), c_all[:])

    # ---- cast w2 to bf16, resident ----
    w2b = []
    for f in range(NF):
        ws = fstage.tile([128, H], f32, tag="fs", name=f"w2s{f}")
        nc.sync.dma_start(ws[:], w2[f * 128:(f + 1) * 128, :])
        wb = w2bp.tile([128, H], bf16, name=f"w2b{f}")
        nc.vector.tensor_copy(wb[:], ws[:])
        w2b.append(wb)

    # ---- per-half compaction -> gather -> transpose ----
    xtg = xtgp.tile([128, NK, NHALF * CAP], f32r, name="xtg")
    cg4 = constp.tile([128, NST], f32, name="cg4")
    gidx = []   # per half: [128, CAP//128] int32 global xpad row ids (0 = pad)
    sidx = []   # per half: [128, CAP//128] int32 local y_h row ids
    for h in range(NHALF):
        # mask row, token-major
        nc.sync.dma_start(
            md[h][:].rearrange("a (t p) -> p (a t)", p=128),
            m_all[:, h * 8:(h + 1) * 8])
        mrow = cmp_.tile([1, TH], f32, tag="mrow", name=f"mrow{h}")
        nc.sync.dma_start(mrow[:], md[h][:])
        rank = cmp_.tile([1, TH], f32, tag="rank", name=f"rank{h}")
        nc.vector.tensor_tensor_scan(rank[:], mrow[:], mrow[:], 0.0,
                                     op0=op.add, op1=op.bypass)
        posf = cmp_.tile([16, TH], f32, tag="posf", name=f"posf{h}")
        nc.vector.memset(posf[:], -1.0)
        nc.vector.scalar_tensor_tensor(posf[0:1, :], rank[:], 1.0, mrow[:],
                                       op0=op.bypass, op1=op.mult)
        nc.vector.tensor_scalar(posf[0:1, :], posf[0:1, :], -1.0, None,
                                op0=op.add)
        okc = cmp_.tile([1, TH], f32, tag="okc", name=f"okc{h}")
        nc.vector.tensor_scalar(okc[:], posf[0:1, :], float(CAP - 1), None,
                                op0=op.is_le)
        nc.vector.scalar_tensor_tensor(posf[0:1, :], posf[0:1, :], 1.0,
                                       okc[:], op0=op.add, op1=op.mult)
        nc.vector.tensor_scalar(posf[0:1, :], posf[0:1, :], -1.0, None,
                                op0=op.add)
        posi = cmp_.tile([16, TH], dt.int16, tag="posi", name=f"posi{h}")
        nc.vector.tensor_copy(posi[:], posf[:])
        vals = cmp_.tile([16, TH], dt.int16, tag="vals", name=f"vals{h}")
        nc.gpsimd.iota(vals[:], pattern=[[1, TH]], base=h * TH + 1,
                       channel_multiplier=0)
        idx16 = cmp_.tile([16, CAP], dt.int16, tag="idx16", name=f"idx16_{h}")
        with tc.tile_critical():
            nc.gpsimd.load_library(library_config.local_scatter)
            nc.gpsimd.local_scatter(idx16[:], vals[:], posi[:], channels=16,
                                    num_elems=CAP, num_idxs=TH)
            nc.gpsimd.load_library(library_config.standard)
        idx32 = cmp_.tile([16, CAP], dt.int32, tag="idx32", name=f"idx32_{h}")
        nc.vector.tensor_copy(idx32[:], idx16[:])
        nc.sync.dma_start(idxd[h][:], idx32[0:1, :])
        gi = cmp_.tile([128, CAP // 128], dt.int32, name=f"gi{h}")
        nc.sync.dma_start(gi[:], idxd[h][:].rearrange("a (t p) -> p (a t)", p=128))
        gidx.append(gi)
        if h == 0:
            sidx.append(gi)
        else:
            si = cmp_.tile([128, CAP // 128], dt.int32, name=f"si{h}")
            nc.vector.tensor_scalar(si[:], gi[:], -h * TH, None, op0=op.add)
            sidx.append(si)
        for tl_ in range(CAP // 128):
            st = h * (CAP // 128) + tl_
            # gather coefficient column for this slot tile
            nc.gpsimd.indirect_dma_start(
                out=cg4[:, st:st + 1], out_offset=None, in_=cpad[:],
                in_offset=bass.IndirectOffsetOnAxis(ap=gi[:, tl_:tl_ + 1],
                                                    axis=0))
            # gather token rows
            g = gath.tile([128, H], f32, tag="g", name=f"g{st}")
            nc.gpsimd.indirect_dma_start(
                out=g[:], out_offset=None, in_=xpad,
                in_offset=bass.IndirectOffsetOnAxis(ap=gi[:, tl_:tl_ + 1],
                                                    axis=0))
            for k in range(NK):
                pt = mpsum.tile([128, 128], f32, tag="mp", name=f"pt{st}_{k}")
                nc.tensor.transpose(pt[:], g[:, k * 128:(k + 1) * 128],
                                    ident[:])
                nc.vector.tensor_copy(
                    xtg[:, k, st * 128:(st + 1) * 128], pt[:])

    # ---- M1: gate/up in f32r over all gathered slots ----
    NSLOT = NHALF * CAP
    ht = [htp.tile([128, NSLOT], bf16, name=f"ht{f}") for f in range(NF)]
    w1r = w1.bitcast(f32r)   # [NF, 128, NK*128] host-pretiled, contiguous per f
    w3r = w3.bitcast(f32r)
    for f in range(NF):
        w1s = fstage.tile([128, NK, 128], f32r, tag="fr", name=f"w1s{f}")
        nc.sync.dma_start(w1s[:], w1r[f].rearrange("p (k c) -> p k c", k=NK))
        w3s = fstage.tile([128, NK, 128], f32r, tag="fr", name=f"w3s{f}")
        nc.sync.dma_start(w3s[:], w3r[f].rearrange("p (k c) -> p k c", k=NK))
        pg = gpsum.tile([128, NSLOT], f32, tag="pg", name=f"pg{f}")
        for k in range(NK):
            nc.tensor.matmul(pg[:], w1s[:, k, :], xtg[:, k, :],
                             start=(k == 0), stop=(k == NK - 1))
        pu = upsum.tile([128, NSLOT], f32, tag="pu", name=f"pu{f}")
        for k in range(NK):
            nc.tensor.matmul(pu[:], w3s[:, k, :], xtg[:, k, :],
                             start=(k == 0), stop=(k == NK - 1))
        gs = gsp.tile([128, NSLOT], f32, tag="gs", name=f"gs{f}")
        nc.scalar.activation(gs[:], pg[:], Silu)
        nc.vector.tensor_mul(ht[f][:], gs[:], pu[:])

    # ---- M2 + scatter + per-half reduce-scatter ----
    for h in range(NHALF):
        for tl_ in range(CAP // 128):
            st = h * (CAP // 128) + tl_
            tsl = slice(st * 128, (st + 1) * 128)
            ph0 = mpsum.tile([128, 512], f32, tag="mp", name=f"ph0_{st}")
            ph1 = mpsum.tile([128, 512], f32, tag="mp", name=f"ph1_{st}")
            for f in range(NF):
                nc.tensor.matmul(ph0[:], ht[f][:, tsl], w2b[f][:, 0:512],
                                 start=(f == 0), stop=(f == NF - 1))
                nc.tensor.matmul(ph1[:], ht[f][:, tsl], w2b[f][:, 512:H],
                                 start=(f == 0), stop=(f == NF - 1))
            c_ap = cg4[:, st:st + 1]
            o0 = ostage.tile([128, 512], f32, tag="os", name=f"o0_{st}")
            nc.scalar.mul(o0[:], ph0[:], c_ap)
            o1 = ostage.tile([128, 512], f32, tag="os", name=f"o1_{st}")
            nc.scalar.mul(o1[:], ph1[:], c_ap)
            nc.gpsimd.indirect_dma_start(
                out=y_h[h][:], out_offset=bass.IndirectOffsetOnAxis(
                    ap=sidx[h][:, tl_:tl_ + 1], axis=0),
                in_=o0[:], in_offset=None,
                bounds_check=TH, oob_is_err=False)
            nc.gpsimd.indirect_dma_start(
                out=y_h[h][:], out_offset=bass.IndirectOffsetOnAxis(
                    ap=sidx[h][:, tl_:tl_ + 1], axis=0),
                in_=o1[:], in_offset=None, element_offset=512,
                bounds_check=TH, oob_is_err=False)
        nc.gpsimd.collective_compute(
            "ReduceScatter",
            op.add,
            replica_groups=[list(range(N_CORES))],
            ins=[y_h[h][1:TH + 1, :].opt()],
            outs=[y_rs[h].opt()],
        )
        nc.sync.dma_start(
            yshard[h * 128:(h + 1) * 128, :], y_rs[h][:])
    ctx.close()


_CACHED_NC = None


def _get_nc():
    global _CACHED_NC
    if _CACHED_NC is None:
        nc = bacc.Bacc("TRN2", target_bir_lowering=False, debug=False,
                       num_devices=N_CORES)
        xT = nc.dram_tensor("xT", [NT, 128, NK * 128], f32,
                            kind="ExternalInput").ap()
        xpad = nc.dram_tensor("xpad", [T + 1, H], f32, kind="ExternalInput").ap()
        gw = nc.dram_tensor("gw", [H, E], f32, kind="ExternalInput").ap()
        w1 = nc.dram_tensor("w1", [NF, 128, NK * 128], f32,
                            kind="ExternalInput").ap()
        w3 = nc.dram_tensor("w3", [NF, 128, NK * 128], f32,
                            kind="ExternalInput").ap()
        w2 = nc.dram_tensor("w2", [F, H], f32, kind="ExternalInput").ap()
        esel = nc.dram_tensor("esel", [128, NT, E], f32,
                              kind="ExternalInput").ap()
        yshard = nc.dram_tensor("yshard", [T // N_CORES, H], f32,
                                kind="ExternalOutput").ap()
        rlog = nc.dram_tensor("rlog", [T, E], f32, kind="ExternalOutput").ap()
        with tile.TileContext(nc) as tc:
            _build_kernel(tc, xT, xpad, gw, w1, w3, w2, esel, yshard, rlog)
        nc.compile()
        _CACHED_NC = nc
    return _CACHED_NC


def _run(hidden_states, gate_w, w1, w3, w2, trace=False):
    nc = _get_nc()
    x = np.ascontiguousarray(
        np.asarray(hidden_states, dtype=np.float32).reshape(T, H))
    xTf = np.ascontiguousarray(x.T)   # [H, T]
    xT = np.ascontiguousarray(
        xTf.reshape(NK, 128, NT, 128).transpose(2, 1, 0, 3)
        .reshape(NT, 128, NK * 128))
    xpad = np.zeros((T + 1, H), np.float32)
    xpad[1:] = x
    gate_w = np.ascontiguousarray(np.asarray(gate_w, dtype=np.float32))
    eye = np.eye(E, dtype=np.float32)

    def tile_w(w):
        # [H, F] -> [NF, 128p, NK k, 128 fc] so each f-tile load is contiguous
        a = np.asarray(w, dtype=np.float32).reshape(NK, 128, NF, 128)
        return np.ascontiguousarray(
            a.transpose(2, 1, 0, 3).reshape(NF, 128, NK * 128))

    in_maps = []
    for e in range(N_CORES):
        in_maps.append({
            "xT": xT,
            "xpad": xpad,
            "gw": gate_w,
            "w1": tile_w(w1[e]),
            "w3": tile_w(w3[e]),
            "w2": np.ascontiguousarray(np.asarray(w2[e], dtype=np.float32)),
            "esel": np.ascontiguousarray(
                np.broadcast_to(eye[e], (128, NT, E)).astype(np.float32)),
        })
    res = run_bass_kernel_spmd(nc, in_maps, core_ids=list(range(N_CORES)),
                               trace=trace)
    # core i's yshard rows [h*128:(h+1)*128] are global rows h*1024 + i*128 ..
    final = np.empty((T, H), np.float32)
    for e in range(N_CORES):
        sh = res.results[e]["yshard"]
        for h in range(NHALF):
            final[h * TH + e * 128:h * TH + (e + 1) * 128] = \
                sh[h * 128:(h + 1) * 128]
    rlog = res.results[0]["rlog"]
    out = (final.reshape(B, S, H).astype(np.float32),
           rlog.reshape(B, S, E).astype(np.float32))
    return out, res


def kernel(hidden_states, gate_w, w1, w3, w2):
    out, _ = _run(hidden_states, gate_w, w1, w3, w2, trace=False)
    return out
